# revision 10
# baseline (speedup 1.0000x reference)
"""CrossTransformerLayer on 8 TRN2 NeuronCores.

Sharding: core c -> (batch b = c//2, q-half = c%2). Each core computes its
512 query rows of its batch end-to-end (k/v over the full 1024-token x1
sequence), so no cross-core collectives are needed. The host slices inputs,
pre-transposes the attention bias to [head, k, q] (so the device adds it to
scores^T tiles with contiguous DMAs), and reassembles the 8 output slices.

Device-side dataflow (per core), all matmuls fp32r (full-rate, ~fp32 prec):
  LN(x1) -> y1 -> y1^T (PE transpose);  LN(x2h) -> y2 -> y2^T
  q^T = wq^T-chunks @ y2^T   k^T = wk^T-chunks @ y1^T    (feature-major)
  v   = y1^T-chunks @ wv                                  (token-major, with
                                                           ones column per head)
  scores^T[k,q] = k^T.T @ q^T;  p = exp(scores*scale + bias^T)  (no max-sub:
        scores*0.125+bias ~ N(0,1.1), |max| < ~7, exp is fp32-safe)
  [o^T | rowsum] = [v|1].T @ p   -> o^T = o^T * (1/rowsum)      (feature-major)
  x = x2h + o^T.T-chunks @ wo + bo;  LN(x) -> y3 -> y3^T
  h^T = w1-chunks @ y3^T;  h = gelu(h^T + b1)  (b1 is per-partition here)
  out = x + h^T.T-chunks @ w2 + b2
"""

import sys

sys.path.insert(0, "/opt/trn_rl_repo")

from contextlib import ExitStack

import numpy as np

import concourse.bass as bass
import concourse.tile as tile
from concourse import bacc, mybir
from concourse.masks import make_identity

F32 = mybir.dt.float32
MM_DT = mybir.dt.float32r  # tensor-engine compute dtype (bitcast of f32)

B = 4
S = 1024   # full (k) sequence
Sq = 512   # query rows per core
H = 1024
NH = 16
Dh = 64    # head dim
FF = 4096
P = 128
NKT = S // P    # 8 k-token tiles
NFC = H // P    # 8 feature chunks
NTC = Sq // P   # 4 q-token tiles
NFFC = FF // P  # 32 ff chunks
EPS = 1e-5
SCALE = float(Dh) ** -0.5
AF = mybir.ActivationFunctionType
OP = mybir.AluOpType


def _pbcast(ap: bass.AP, parts: int) -> bass.AP:
    """[.., N] access pattern -> [parts, .., N] with partition step 0."""
    return bass.AP(
        tensor=ap.tensor,
        offset=ap.offset,
        ap=[[0, parts]] + [list(d) for d in ap.ap],
    )


def _layer_norm(nc, pool, y_out, x_in, g_b, b_b, eps_t):
    """y = (x - mean)/sqrt(var+eps) * g + b on a [128, H] token-major tile."""
    stats = pool.tile([P, 2, 6], F32, tag="ln_stats")
    nc.vector.bn_stats(stats[:, 0, :], x_in[:, 0:512])
    nc.vector.bn_stats(stats[:, 1, :], x_in[:, 512:1024])
    mv = pool.tile([P, 2], F32, tag="ln_mv")
    nc.vector.bn_aggr(mv, stats)
    std = pool.tile([P, 1], F32, tag="ln_std")
    nc.scalar.activation(std, mv[:, 1:2], AF.Sqrt, bias=eps_t, scale=1.0)
    rstd = pool.tile([P, 1], F32, tag="ln_rstd")
    nc.vector.reciprocal(rstd, std)
    nc.vector.tensor_scalar(
        y_out, x_in, mv[:, 0:1], rstd, op0=OP.subtract, op1=OP.mult
    )
    nc.vector.tensor_mul(y_out, y_out, g_b)
    nc.vector.tensor_add(y_out, y_out, b_b)


def build_program():
    nc = bacc.Bacc("TRN2", target_bir_lowering=False, debug=False)

    x1_d = nc.dram_tensor("x1", (S, H), F32, kind="ExternalInput")
    x2h_d = nc.dram_tensor("x2h", (Sq, H), F32, kind="ExternalInput")
    biasT_d = nc.dram_tensor("biasT", (NH, S, Sq), F32, kind="ExternalInput")
    wq_d = nc.dram_tensor("wq", (H, H), F32, kind="ExternalInput")
    wk_d = nc.dram_tensor("wk", (H, H), F32, kind="ExternalInput")
    wv_d = nc.dram_tensor("wv", (H, H), F32, kind="ExternalInput")
    wo_d = nc.dram_tensor("wo", (H, H), F32, kind="ExternalInput")
    bq_d = nc.dram_tensor("bq_pc", (P, NFC), F32, kind="ExternalInput")
    bk_d = nc.dram_tensor("bk_pc", (P, NFC), F32, kind="ExternalInput")
    bv_d = nc.dram_tensor("bv", (H,), F32, kind="ExternalInput")
    bo_d = nc.dram_tensor("bo", (H,), F32, kind="ExternalInput")
    w1_d = nc.dram_tensor("w1", (H, FF), F32, kind="ExternalInput")
    b1_d = nc.dram_tensor("b1_pc", (P, NFFC), F32, kind="ExternalInput")
    w2_d = nc.dram_tensor("w2", (FF, H), F32, kind="ExternalInput")
    b2_d = nc.dram_tensor("b2", (H,), F32, kind="ExternalInput")
    ln1g_d = nc.dram_tensor("ln1_g", (H,), F32, kind="ExternalInput")
    ln1b_d = nc.dram_tensor("ln1_b", (H,), F32, kind="ExternalInput")
    ln2g_d = nc.dram_tensor("ln2_g", (H,), F32, kind="ExternalInput")
    ln2b_d = nc.dram_tensor("ln2_b", (H,), F32, kind="ExternalInput")
    lnfg_d = nc.dram_tensor("lnf_g", (H,), F32, kind="ExternalInput")
    lnfb_d = nc.dram_tensor("lnf_b", (H,), F32, kind="ExternalInput")
    out_d = nc.dram_tensor("out", (Sq, H), F32, kind="ExternalOutput")

    def _mm(out, lhsT, rhs, **kw):
        nc.tensor.matmul(out, lhsT.bitcast(MM_DT), rhs.bitcast(MM_DT), **kw)

    def _w_quarter(pool, wd, quarter):
        """Load a [128, kc, 256] column-quarter of a (H, H) weight."""
        w_sb = pool.tile([P, NFC, 256], F32, tag="w", name="w_sb")
        nc.sync.dma_start(
            w_sb.bitcast(MM_DT),
            wd[:, quarter * 256:(quarter + 1) * 256].rearrange(
                "(kc p) f -> p kc f", p=P
            ).bitcast(MM_DT),
        )
        return w_sb

    with tile.TileContext(nc) as tc, ExitStack() as top:
        persist = top.enter_context(tc.tile_pool(name="persist", bufs=1))
        ident = persist.tile([P, P], F32, tag="ident")
        make_identity(nc, ident)
        x_sb = persist.tile([P, NTC, H], F32, tag="x")      # [p, tc, f], ph 4-7
        oT = persist.tile([P, NFC, Sq], F32, tag="oT")      # [p, fc, q], ph 3-4

        with tc.tile_pool(name="qkv", bufs=1) as qkvp:      # phases 1-3
            qT = qkvp.tile([P, NFC, Sq], F32, tag="qT")         # [p, fc, q]
            kT = qkvp.tile([P, NFC, S], F32, tag="kT")          # [p, fc, k]
            v_aug = qkvp.tile([P, NKT, NH * 65], F32, tag="vaug")

            # ------------ Phase 1+2: LN, transpose, QKV projections ---------
            with tc.tile_pool(name="y12", bufs=1) as y12:
                y1T = y12.tile([P, NFC, S], F32, tag="y1T")
                y2T = y12.tile([P, NFC, Sq], F32, tag="y2T")

                with (
                    tc.tile_pool(name="ph1", bufs=4) as ph1,
                    tc.tile_pool(name="ph1w", bufs=3) as ph1w,
                    tc.tile_pool(name="ph1c", bufs=1) as ph1c,
                    tc.tile_pool(
                        name="ph1ps", bufs=4, space=bass.MemorySpace.PSUM
                    ) as ph1ps,
                ):
                    eps_t = ph1c.tile([P, 1], F32, tag="eps")
                    nc.vector.memset(eps_t, EPS)
                    ln1g_b = ph1c.tile([P, H], F32, tag="ln1g")
                    ln1b_b = ph1c.tile([P, H], F32, tag="ln1b")
                    ln2g_b = ph1c.tile([P, H], F32, tag="ln2g")
                    ln2b_b = ph1c.tile([P, H], F32, tag="ln2b")
                    nc.sync.dma_start(ln1g_b, _pbcast(ln1g_d[:], P))
                    nc.sync.dma_start(ln1b_b, _pbcast(ln1b_d[:], P))
                    nc.sync.dma_start(ln2g_b, _pbcast(ln2g_d[:], P))
                    nc.sync.dma_start(ln2b_b, _pbcast(ln2b_d[:], P))

                    for t in range(NKT):  # x1 -> y1 -> y1T
                        xt = ph1.tile([P, H], F32, tag="xt")
                        nc.sync.dma_start(xt, x1_d[t * P:(t + 1) * P, :])
                        yt = ph1w.tile([P, H], F32, tag="yt")
                        _layer_norm(nc, ph1, yt, xt, ln1g_b, ln1b_b, eps_t)
                        for fc in range(NFC):
                            pt = ph1ps.tile([P, P], F32, tag="tr")
                            nc.tensor.transpose(
                                pt, yt[:, fc * P:(fc + 1) * P], ident
                            )
                            nc.any.tensor_copy(y1T[:, fc, t * P:(t + 1) * P].bitcast(MM_DT), pt)

                    for t in range(NTC):  # x2h -> y2 -> y2T
                        xt = ph1.tile([P, H], F32, tag="xt")
                        nc.sync.dma_start(xt, x2h_d[t * P:(t + 1) * P, :])
                        yt = ph1w.tile([P, H], F32, tag="yt")
                        _layer_norm(nc, ph1, yt, xt, ln2g_b, ln2b_b, eps_t)
                        for fc in range(NFC):
                            pt = ph1ps.tile([P, P], F32, tag="tr")
                            nc.tensor.transpose(
                                pt, yt[:, fc * P:(fc + 1) * P], ident
                            )
                            nc.any.tensor_copy(y2T[:, fc, t * P:(t + 1) * P].bitcast(MM_DT), pt)

                # --------- QKV projections ---------
                with (
                    tc.tile_pool(name="wload", bufs=2) as wpool,
                    tc.tile_pool(name="vecs", bufs=1) as vecs,
                    tc.tile_pool(
                        name="ph2ps", bufs=3, space=bass.MemorySpace.PSUM
                    ) as ps2,
                ):
                    bq_sb = vecs.tile([P, NFC], F32, tag="bq")
                    bk_sb = vecs.tile([P, NFC], F32, tag="bk")
                    bv_b = vecs.tile([P, H], F32, tag="bvb")
                    nc.sync.dma_start(bq_sb, bq_d[:, :])
                    nc.sync.dma_start(bk_sb, bk_d[:, :])
                    nc.sync.dma_start(bv_b, _pbcast(bv_d[:], P))

                    # ones columns of v_aug (slot 64 of each head); memset
                    # can't write fp32r, so memset f32 then copy-broadcast
                    ones_view = v_aug[:, :, :].rearrange(
                        "p t (h j) -> p t h j", j=65
                    )[:, :, :, 64:65]
                    ones_src = vecs.tile([P, 1], F32, tag="ones")
                    nc.vector.memset(ones_src, 1.0)
                    osa = ones_src[:, 0:1]
                    nc.vector.tensor_copy(
                        ones_view.bitcast(MM_DT),
                        bass.AP(
                            tensor=osa.tensor,
                            offset=osa.offset,
                            ap=[list(osa.ap[0]), [0, NKT], [0, NH], [1, 1]],
                        ),
                    )

                    # q^T[fo, :] = sum_kc wq[kc, fo].T @ y2T[kc]  (+bq)
                    for quarter in range(4):
                        wq_sb = _w_quarter(wpool, wq_d, quarter)
                        for fo_i in range(2):
                            fo = quarter * 2 + fo_i
                            ps = ps2.tile([P, Sq], F32, tag="mm")
                            for kc in range(NFC):
                                _mm(
                                    ps,
                                    wq_sb[:, kc, fo_i * P:(fo_i + 1) * P],
                                    y2T[:, kc, :],
                                    start=(kc == 0),
                                    stop=(kc == NFC - 1),
                                )
                            nc.scalar.activation(
                                qT[:, fo, :].bitcast(MM_DT), ps, AF.Identity,
                                bias=bq_sb[:, fo:fo + 1], scale=1.0,
                            )

                    # k^T[fo, nt] = sum_kc wk[kc, fo].T @ y1T[kc, nt]  (+bk)
                    for quarter in range(4):
                        wk_sb = _w_quarter(wpool, wk_d, quarter)
                        for fo_i in range(2):
                            fo = quarter * 2 + fo_i
                            for nt in range(2):
                                ps = ps2.tile([P, 512], F32, tag="mm")
                                for kc in range(NFC):
                                    _mm(
                                        ps,
                                        wk_sb[:, kc, fo_i * P:(fo_i + 1) * P],
                                        y1T[:, kc, nt * 512:(nt + 1) * 512],
                                        start=(kc == 0),
                                        stop=(kc == NFC - 1),
                                    )
                                nc.scalar.activation(
                                    kT[:, fo, nt * 512:(nt + 1) * 512].bitcast(MM_DT),
                                    ps,
                                    AF.Identity, bias=bk_sb[:, fo:fo + 1],
                                    scale=1.0,
                                )

                    # v[t, q] = sum_kc y1T[kc, t].T @ wv[kc, q]  (+bv) -> v_aug
                    for quarter in range(4):
                        wv_sb = _w_quarter(wpool, wv_d, quarter)
                        for t in range(NKT):
                            ps = ps2.tile([P, 256], F32, tag="mmv")
                            for kc in range(NFC):
                                _mm(
                                    ps,
                                    y1T[:, kc, t * P:(t + 1) * P],
                                    wv_sb[:, kc, :],
                                    start=(kc == 0),
                                    stop=(kc == NFC - 1),
                                )
                            dst = v_aug[
                                :, t, quarter * 4 * 65:(quarter * 4 + 4) * 65
                            ].rearrange("p (h j) -> p h j", j=65)[:, :, 0:64]
                            nc.vector.tensor_tensor(
                                out=dst.bitcast(MM_DT),
                                in0=ps.rearrange("p (h j) -> p h j", j=64),
                                in1=bv_b[
                                    :, quarter * 256:(quarter + 1) * 256
                                ].rearrange("p (h j) -> p h j", j=64),
                                op=OP.add,
                            )

            # ---------------- Phase 3: attention ----------------
            with (
                tc.tile_pool(name="bias_s", bufs=8) as bpool,
                tc.tile_pool(name="expp", bufs=2) as epool,
                tc.tile_pool(name="scr", bufs=4) as scr,
                tc.tile_pool(name="rin", bufs=2) as rpool,
                tc.tile_pool(
                    name="sc_ps", bufs=4, space=bass.MemorySpace.PSUM
                ) as scps,
                tc.tile_pool(
                    name="o_ps", bufs=2, space=bass.MemorySpace.PSUM
                ) as ops,
            ):
                for h in range(NH):
                    hp = (h % 2) * Dh
                    fc = h // 2
                    o_ps = ops.tile([65, Sq], F32, tag="o", name="o_ps")
                    e_t = epool.tile([P, NKT, Sq], F32, tag="expT", name="e_t")
                    for kt in range(NKT):
                        sc_ps = scps.tile([P, Sq], F32, tag="sc", name="sc_ps")
                        _mm(
                            sc_ps,
                            kT[hp:hp + Dh, fc, kt * P:(kt + 1) * P],
                            qT[hp:hp + Dh, fc, :],
                            start=True, stop=True,
                        )
                        bt = bpool.tile([P, Sq], F32, tag="bt", name="bt")
                        nc.sync.dma_start(bt, biasT_d[h, kt * P:(kt + 1) * P, :])
                        st = scr.tile([P, Sq], F32, tag="st", name="st")
                        nc.vector.scalar_tensor_tensor(
                            out=st, in0=sc_ps, scalar=SCALE, in1=bt,
                            op0=OP.mult, op1=OP.add,
                        )
                        nc.scalar.activation(e_t[:, kt, :].bitcast(MM_DT), st, AF.Exp)
                        _mm(
                            o_ps,
                            v_aug[:, kt, h * 65:(h + 1) * 65],
                            e_t[:, kt, :],
                            start=(kt == 0), stop=(kt == NKT - 1),
                        )
                    rinv = rpool.tile([1, Sq], F32, tag="rinv", name="rinv")
                    nc.vector.reciprocal(rinv, o_ps[64:65, :])
                    rb = rpool.tile([Dh, Sq], F32, tag="rb", name="rb")
                    nc.gpsimd.partition_broadcast(rb, rinv[0:1, :])
                    nc.vector.tensor_tensor(
                        out=oT[hp:hp + Dh, fc, :].bitcast(MM_DT),
                        in0=o_ps[0:64, :], in1=rb,
                        op=OP.mult,
                    )

        # ---------------- Phase 4: output projection + residual -------------
        with (
            tc.tile_pool(name="ph4w", bufs=1) as w4pool,
            tc.tile_pool(name="ph4x", bufs=3) as ph4x,
            tc.tile_pool(name="ph4c", bufs=1) as ph4c,
            tc.tile_pool(name="ph4ps", bufs=3, space=bass.MemorySpace.PSUM) as ps4,
        ):
            bo_b = ph4c.tile([P, H], F32, tag="bob")
            nc.sync.dma_start(bo_b, _pbcast(bo_d[:], P))
            wo_sbs = []
            for half in range(2):
                wo_sb = w4pool.tile(
                    [P, NFC, 512], F32, tag=f"w{half}", name="wo_sb"
                )
                nc.sync.dma_start(
                    wo_sb.bitcast(MM_DT),
                    wo_d[:, half * 512:(half + 1) * 512].rearrange(
                        "(kc p) f -> p kc f", p=P
                    ).bitcast(MM_DT),
                )
                wo_sbs.append(wo_sb)
            for t in range(NTC):
                x2t = ph4x.tile([P, H], F32, tag="x2t", name="x2t")
                nc.sync.dma_start(x2t, x2h_d[t * P:(t + 1) * P, :])
                for half in range(2):
                    wo_sb = wo_sbs[half]
                    ps = ps4.tile([P, 512], F32, tag="mm", name="ps")
                    for kc in range(NFC):
                        _mm(
                            ps,
                            oT[:, kc, t * P:(t + 1) * P],
                            wo_sb[:, kc, :],
                            start=(kc == 0), stop=(kc == NFC - 1),
                        )
                    xs = x_sb[:, t, half * 512:(half + 1) * 512]
                    nc.vector.tensor_tensor(
                        out=xs, in0=ps,
                        in1=x2t[:, half * 512:(half + 1) * 512], op=OP.add,
                    )
                    nc.vector.tensor_tensor(
                        out=xs, in0=xs,
                        in1=bo_b[:, half * 512:(half + 1) * 512], op=OP.add,
                    )

        # ---------------- Phase 5+6+7: final LN + FFN ----------------
        with tc.tile_pool(name="hT", bufs=1) as hTp:
            hT = hTp.tile([P, NFFC, Sq], F32, tag="hT")

            with tc.tile_pool(name="y3", bufs=1) as y3p:
                y3T = y3p.tile([P, NFC, Sq], F32, tag="y3T")
                with (
                    tc.tile_pool(name="ph5", bufs=4) as ph5,
                    tc.tile_pool(name="ph5w", bufs=2) as ph5w,
                    tc.tile_pool(name="ph5c", bufs=1) as ph5c,
                    tc.tile_pool(
                        name="ph5ps", bufs=4, space=bass.MemorySpace.PSUM
                    ) as ph5ps,
                ):
                    eps_t = ph5c.tile([P, 1], F32, tag="eps")
                    nc.vector.memset(eps_t, EPS)
                    lnfg_b = ph5c.tile([P, H], F32, tag="lnfg")
                    lnfb_b = ph5c.tile([P, H], F32, tag="lnfb")
                    nc.sync.dma_start(lnfg_b, _pbcast(lnfg_d[:], P))
                    nc.sync.dma_start(lnfb_b, _pbcast(lnfb_d[:], P))
                    for t in range(NTC):
                        yt = ph5w.tile([P, H], F32, tag="yt")
                        _layer_norm(
                            nc, ph5, yt, x_sb[:, t, :], lnfg_b, lnfb_b, eps_t
                        )
                        for fc in range(NFC):
                            pt = ph5ps.tile([P, P], F32, tag="tr")
                            nc.tensor.transpose(
                                pt, yt[:, fc * P:(fc + 1) * P], ident
                            )
                            nc.any.tensor_copy(y3T[:, fc, t * P:(t + 1) * P].bitcast(MM_DT), pt)

                # FFN1 + gelu -> hT fully resident in SBUF
                with (
                    tc.tile_pool(name="w1l", bufs=3) as w1pool,
                    tc.tile_pool(name="b1l", bufs=1) as b1pool,
                    tc.tile_pool(
                        name="f1ps", bufs=3, space=bass.MemorySpace.PSUM
                    ) as f1ps,
                ):
                    b1_sb = b1pool.tile([P, NFFC], F32, tag="b1")
                    nc.sync.dma_start(b1_sb, b1_d[:, :])
                    for ffc in range(NFFC):
                        w1c = w1pool.tile([P, NFC, P], F32, tag="w1c", name="w1c")
                        nc.sync.dma_start(
                            w1c.bitcast(MM_DT),
                            w1_d[:, ffc * P:(ffc + 1) * P].rearrange(
                                "(kc p) c -> p kc c", p=P
                            ).bitcast(MM_DT),
                        )
                        ps = f1ps.tile([P, Sq], F32, tag="mm", name="ps")
                        for kc in range(NFC):
                            _mm(
                                ps, w1c[:, kc, :], y3T[:, kc, :],
                                start=(kc == 0), stop=(kc == NFC - 1),
                            )
                        nc.scalar.activation(
                            hT[:, ffc, :].bitcast(MM_DT), ps, AF.Gelu,
                            bias=b1_sb[:, ffc:ffc + 1], scale=1.0,
                        )

            # FFN2: single pass, full 8-bank PSUM accumulation
            with (
                tc.tile_pool(name="w2l", bufs=4) as w2pool,
                tc.tile_pool(name="ph7c", bufs=1) as ph7c,
                tc.tile_pool(name="outp", bufs=2) as outp,
                tc.tile_pool(name="f2ps", bufs=1, space=bass.MemorySpace.PSUM) as f2ps,
            ):
                b2_b = ph7c.tile([P, H], F32, tag="b2b")
                nc.sync.dma_start(b2_b, _pbcast(b2_d[:], P))
                acc = [
                    f2ps.tile([P, H], F32, tag=f"acc{t}", name=f"acc{t}")
                    for t in range(NTC)
                ]
                for ffc in range(NFFC):
                    w2c = w2pool.tile([P, H], F32, tag="w2c", name="w2c")
                    nc.sync.dma_start(w2c.bitcast(MM_DT), w2_d[ffc * P:(ffc + 1) * P, :].bitcast(MM_DT))
                    for t in range(NTC):
                        for nt in range(2):
                            _mm(
                                acc[t][:, nt * 512:(nt + 1) * 512],
                                hT[:, ffc, t * P:(t + 1) * P],
                                w2c[:, nt * 512:(nt + 1) * 512],
                                start=(ffc == 0), stop=(ffc == NFFC - 1),
                            )
                for t in range(NTC):
                    ot = outp.tile([P, H], F32, tag="ot", name="ot")
                    nc.vector.tensor_tensor(
                        out=ot, in0=acc[t], in1=x_sb[:, t, :], op=OP.add
                    )
                    nc.vector.tensor_tensor(out=ot, in0=ot, in1=b2_b, op=OP.add)
                    nc.sync.dma_start(out_d[t * P:(t + 1) * P, :], ot)

    nc.compile()
    return nc


_CACHE: dict = {}


def _get_program():
    if "nc" not in _CACHE:
        _CACHE["nc"] = build_program()
    return _CACHE["nc"]


def _make_in_maps(inputs: dict) -> list[dict]:
    f32 = lambda a: np.ascontiguousarray(np.asarray(a, dtype=np.float32))
    x1 = f32(inputs["x1"])
    x2 = f32(inputs["x2"])
    attn_bias = f32(inputs["attn_bias"])
    shared = {
        "wq": f32(inputs["wq"]),
        "wk": f32(inputs["wk"]),
        "wv": f32(inputs["wv"]),
        "wo": f32(inputs["wo"]),
        "bq_pc": f32(np.asarray(inputs["bq"]).reshape(NFC, P).T),
        "bk_pc": f32(np.asarray(inputs["bk"]).reshape(NFC, P).T),
        "bv": f32(inputs["bv"]),
        "bo": f32(inputs["bo"]),
        "w1": f32(inputs["w1"]),
        "b1_pc": f32(np.asarray(inputs["b1"]).reshape(NFFC, P).T),
        "w2": f32(inputs["w2"]),
        "b2": f32(inputs["b2"]),
        "ln1_g": f32(inputs["ln1_g"]),
        "ln1_b": f32(inputs["ln1_b"]),
        "ln2_g": f32(inputs["ln2_g"]),
        "ln2_b": f32(inputs["ln2_b"]),
        "lnf_g": f32(inputs["lnf_g"]),
        "lnf_b": f32(inputs["lnf_b"]),
    }
    in_maps = []
    for c in range(8):
        b, half = c // 2, c % 2
        q0 = half * Sq
        in_maps.append(
            {
                "x1": x1[b],
                "x2h": np.ascontiguousarray(x2[b, q0:q0 + Sq]),
                "biasT": np.ascontiguousarray(
                    attn_bias[b, :, q0:q0 + Sq, :].transpose(0, 2, 1)
                ),
                **shared,
            }
        )
    return in_maps


def _assemble(results: list[dict]) -> np.ndarray:
    out = np.empty((B, S, H), np.float32)
    for c in range(8):
        b, half = c // 2, c % 2
        out[b, half * Sq:(half + 1) * Sq] = results[c]["out"]
    return out


def run(inputs: dict, **run_kwargs):
    from concourse.bass_utils import run_bass_kernel_spmd

    nc = _get_program()
    in_maps = _make_in_maps(inputs)
    res = run_bass_kernel_spmd(nc, in_maps, core_ids=list(range(8)), **run_kwargs)
    return _assemble(res.results), res


def kernel(**inputs) -> np.ndarray:
    out, _ = run(inputs)
    return out
